# revision 7
# baseline (speedup 1.0000x reference)
"""AtomPooling Trainium2 kernel: segment-softmax attention pooling.

reference semantics (uniform contiguous 200-atom segments):
    scores = x @ w + b ; per-segment softmax ; out[s] = sum att * x

Design (per NeuronCore, 8-way data parallel over segments):
  - shard: core i gets segments [250*i, 250*i+250) -> 50_000 atoms,
    padded with zeros to 51_200 atoms = 16 blocks x 3200 atoms.
  - block = 3200 atoms = 25 tiles of 128 partitions = 16 segments.
    atom layout inside a block is interleaved: atom = 3200*b + 25*p + T
    so each partition's 25 atoms are contiguous in DRAM (fat DMA descriptors).
  - x is cast f32->bf16 during the (SWDGE) DMA.
  - xT (d on partitions) produced by one SBUF->SBUF xbar DMA transpose per
    block; scores = matmul(lhsT=xT_chunk[128d,128atoms], rhs=w_chunk[128,1])
    accumulated over the two 128-d chunks -> PSUM [128 atoms, 1] per tile.
  - softmax: +b dropped (softmax shift invariant), no max subtraction
    (scores ~ N(0,1), exp safe). e = exp(scores) on ScalarE, bf16.
  - A[p, j] = e[p] * mask[p, j] (mask = one-hot segment slot, host data).
  - pooled: U[16,256] += A^T @ x_tile ; denom[16,1] += A^T @ ones
    accumulated over the 25 tiles of a block in PSUM.
  - out = U * (1/denom) on DVE, DMA'd out. Host concatenates + trims pads.
"""

import numpy as np

# ---- hardcoded problem config (nn_AtomPooling: 400k x 256, 2000 x 200) ----
N_ATOMS = 400_000
D = 256
N_SEG = 2_000
SEG_LEN = 200
N_CORES = 8

P = 128                 # partitions
TPB = 25                # tiles per block
SPB = 16                # segments per block
APB = P * TPB           # atoms per block = 3200
BLOCKS = 16             # blocks per core
ATOMS_CORE = APB * BLOCKS   # 51200 (padded; 50000 real)
SEGS_CORE = SPB * BLOCKS    # 256   (250 real)
REAL_ATOMS = N_ATOMS // N_CORES
REAL_SEGS = N_SEG // N_CORES

_CACHE = {}


def _build_nc():
    import concourse.bacc as bacc
    import concourse.mybir as mybir
    from concourse.tile import TileContext

    fp32 = mybir.dt.float32
    bf16 = mybir.dt.bfloat16

    nc = bacc.Bacc(None, target_bir_lowering=False, debug=False)
    x_ext = nc.declare_dram_parameter("x", [ATOMS_CORE, D], fp32, isOutput=False)
    w_ext = nc.declare_dram_parameter("w", [D], fp32, isOutput=False)
    m_ext = nc.declare_dram_parameter("masks", [TPB, P, SPB], fp32, isOutput=False)
    out_ext = nc.declare_dram_parameter("out", [SEGS_CORE, D], fp32, isOutput=True)

    with TileContext(nc) as tc:
        with (
            tc.tile_pool(name="const", bufs=1) as cpool,
            tc.tile_pool(name="xblk", bufs=3) as xpool,
            tc.tile_pool(name="xt", bufs=3) as xtpool,
            tc.tile_pool(name="at", bufs=8) as apool,
            tc.tile_pool(name="eblk", bufs=3) as epool,
            tc.tile_pool(name="outp", bufs=3) as opool,
            tc.tile_pool(name="ps_s", bufs=2, space="PSUM") as ps_s,
            tc.tile_pool(name="ps_u", bufs=2, space="PSUM") as ps_u,
            tc.tile_pool(name="ps_d", bufs=2, space="PSUM") as ps_d,
        ):
            # ---- constants ----
            w_f32 = cpool.tile([P, 2], fp32)
            nc.scalar.dma_start(out=w_f32[:], in_=w_ext.ap().rearrange("(c p) -> p c", p=P))
            w_sb = cpool.tile([P, 2], bf16)
            nc.vector.tensor_copy(w_sb[:], w_f32[:])

            m_f32 = cpool.tile([P, TPB * SPB], fp32)
            nc.scalar.dma_start(
                out=m_f32[:].rearrange("p (T j) -> p T j", j=SPB),
                in_=m_ext.ap().rearrange("T p j -> p T j"))
            m_sb = cpool.tile([P, TPB * SPB], bf16)
            nc.vector.tensor_copy(m_sb[:], m_f32[:])

            ones_sb = cpool.tile([P, 1], bf16)
            nc.vector.memset(ones_sb[:], 1.0)

            # scratch for dependency-absorbing dummy DMAs (see below)
            scratch_sb = cpool.tile([1, 2], bf16)

            for b in range(BLOCKS):
                # ---- load x block, casting f32 -> bf16 in the DMA ----
                x_blk = xpool.tile([P, TPB * D], bf16)
                src = x_ext.ap()[b * APB:(b + 1) * APB, :].rearrange(
                    "(p T) c -> p (T c)", p=P)
                nc.gpsimd.dma_start(out=x_blk[:], in_=src)

                # ---- transposed copy via xbar DMA (SBUF->SBUF) ----
                # The XPOSE instruction supports few sync-wait slots, so
                # absorb its dependencies (x_blk written, xt slot free) into
                # dummy DMAs issued earlier on the same queue.
                xt_blk = xtpool.tile([P, TPB * 2 * P], bf16)
                nc.vector.tensor_copy(xt_blk[0:1, 0:2], x_blk[0:1, 0:2])
                nc.sync.dma_start(
                    out=xt_blk[:].rearrange("p (t n) -> p t n", n=P),
                    in_=x_blk[:],
                    transpose=True,
                )

                # ---- scores: per tile [128,1] = xT_chunk^T @ w_chunk ----
                scores_ps = ps_s.tile([P, TPB], mybir.dt.float32)
                for T in range(TPB):
                    for c in range(2):
                        nc.tensor.matmul(
                            scores_ps[:, T:T + 1],
                            lhsT=xt_blk[:, (2 * T + c) * P:(2 * T + c + 1) * P],
                            rhs=w_sb[:, c:c + 1],
                            start=(c == 0), stop=(c == 1),
                            skip_group_check=True,
                        )

                # ---- e = exp(scores) (softmax shift skipped; see header) ----
                e_blk = epool.tile([P, TPB], mybir.dt.float32)
                nc.scalar.activation(e_blk[:], scores_ps[:],
                                     mybir.ActivationFunctionType.Exp)

                # ---- pooled + denom accumulation over tiles ----
                U_ps = ps_u.tile([SPB, D], mybir.dt.float32)
                d_ps = ps_d.tile([SPB, 1], mybir.dt.float32)
                for T in range(TPB):
                    a_t = apool.tile([P, SPB], bf16)
                    nc.vector.tensor_scalar_mul(
                        a_t[:], m_sb[:, T * SPB:(T + 1) * SPB], e_blk[:, T:T + 1])
                    nc.tensor.matmul(
                        U_ps[:], lhsT=a_t[:], rhs=x_blk[:, T * D:(T + 1) * D],
                        start=(T == 0), stop=(T == TPB - 1),
                        skip_group_check=True,
                    )
                    nc.tensor.matmul(
                        d_ps[:], lhsT=a_t[:], rhs=ones_sb[:],
                        start=(T == 0), stop=(T == TPB - 1),
                        skip_group_check=True,
                    )

                # ---- out = U / denom ----
                r_sb = opool.tile([SPB, 1], mybir.dt.float32)
                nc.vector.reciprocal(r_sb[:], d_ps[:])
                o_sb = opool.tile([SPB, D], mybir.dt.float32)
                nc.vector.tensor_scalar_mul(o_sb[:], U_ps[:], r_sb[:])
                nc.scalar.dma_start(
                    out=out_ext.ap()[b * SPB:(b + 1) * SPB, :], in_=o_sb[:])

    nc.compile()
    return nc


def _masks_np():
    m = np.zeros((TPB, P, SPB), np.float32)
    a = 25 * np.arange(P)[:, None] + np.arange(TPB)[None, :]   # [p, T]
    slot = a // SEG_LEN
    for T in range(TPB):
        m[T, np.arange(P), slot[:, T]] = 1.0
    return m


def _is_uniform(index_list):
    il = np.asarray(index_list)
    starts = il[:, 0]
    return (il.shape == (N_SEG, 2)
            and np.array_equal(starts, np.arange(N_SEG, dtype=starts.dtype) * SEG_LEN))


def _reference_np(atom_features, index_list, w, b):
    """Pure-numpy fallback matching reference() for non-uniform inputs."""
    x = np.asarray(atom_features, np.float32)
    starts = np.asarray(index_list)[:, 0]
    N = x.shape[0]
    S = starts.shape[0]
    seg_ids = np.searchsorted(starts, np.arange(N), side="right") - 1
    scores = x @ np.asarray(w, np.float32) + np.asarray(b, np.float32)[0]
    smax = np.full(S, -np.inf, np.float32)
    np.maximum.at(smax, seg_ids, scores)
    e = np.exp(scores - smax[seg_ids])
    den = np.zeros(S, np.float32)
    np.add.at(den, seg_ids, e)
    att = e / den[seg_ids]
    out = np.zeros((S, x.shape[1]), np.float32)
    np.add.at(out, seg_ids, att[:, None] * x)
    return out


def kernel(**inputs):
    x = np.ascontiguousarray(np.asarray(inputs["atom_features"], dtype=np.float32))
    index_list = inputs["index_list"]
    w = np.ascontiguousarray(np.asarray(inputs["w"], dtype=np.float32))
    b = inputs["b"]

    if not _is_uniform(index_list):
        return _reference_np(x, index_list, w, b)

    from concourse.bass_utils import run_bass_kernel_spmd

    if "nc" not in _CACHE:
        _CACHE["nc"] = _build_nc()
    nc = _CACHE["nc"]

    masks = _masks_np()
    in_maps = []
    for i in range(N_CORES):
        xp = np.zeros((ATOMS_CORE, D), np.float32)
        xp[:REAL_ATOMS] = x[i * REAL_ATOMS:(i + 1) * REAL_ATOMS]
        in_maps.append({"x": xp, "w": w, "masks": masks})

    res = run_bass_kernel_spmd(nc, in_maps, core_ids=list(range(N_CORES)))
    outs = res.results
    full = np.concatenate([np.asarray(outs[i]["out"])[:REAL_SEGS]
                           for i in range(N_CORES)], axis=0)
    return full.astype(np.float32)


# revision 10
# speedup vs baseline: 200.8660x; 200.8660x over previous
"""AtomPooling Trainium2 kernel: segment-softmax attention pooling.

reference semantics (uniform contiguous 200-atom segments):
    scores = x @ w + b ; per-segment softmax ; out[s] = sum att * x

Design (per NeuronCore, 8-way data parallel over segments):
  - shard: core i gets segments [250*i, 250*i+250) -> 50_000 atoms,
    padded with zeros to 51_200 atoms = 16 blocks x 3200 atoms.
  - block = 3200 atoms = 25 tiles of 128 partitions = 16 segments.
    atom layout inside a block is interleaved: atom = 3200*b + 25*p + T
    so each partition's 25 atoms are contiguous in DRAM (fat DMA descriptors).
  - x is cast f32->bf16 during the (SWDGE) DMA.
  - xT (d on partitions) produced by one SBUF->SBUF xbar DMA transpose per
    block; scores = matmul(lhsT=xT_chunk[128d,128atoms], rhs=w_chunk[128,1])
    accumulated over the two 128-d chunks -> PSUM [128 atoms, 1] per tile.
  - softmax: +b dropped (softmax shift invariant), no max subtraction
    (scores ~ N(0,1), exp safe). e = exp(scores) on ScalarE, bf16.
  - A[p, j] = e[p] * mask[p, j] (mask = one-hot segment slot, host data).
  - pooled: U[16,256] += A^T @ x_tile ; denom[16,1] += A^T @ ones
    accumulated over the 25 tiles of a block in PSUM.
  - out = U * (1/denom) on DVE, DMA'd out. Host concatenates + trims pads.
"""

import numpy as np

# ---- hardcoded problem config (nn_AtomPooling: 400k x 256, 2000 x 200) ----
N_ATOMS = 400_000
D = 256
N_SEG = 2_000
SEG_LEN = 200
N_CORES = 8

P = 128                 # partitions
TPB = 25                # tiles per block
SPB = 16                # segments per block
APB = P * TPB           # atoms per block = 3200
BLOCKS = 16             # blocks per core
ATOMS_CORE = APB * BLOCKS   # 51200 (padded; 50000 real)
SEGS_CORE = SPB * BLOCKS    # 256   (250 real)
REAL_ATOMS = N_ATOMS // N_CORES
REAL_SEGS = N_SEG // N_CORES

_CACHE = {}


def _build_nc(reps=1):
    import concourse.bacc as bacc
    import concourse.mybir as mybir
    from concourse.tile import TileContext
    from contextlib import nullcontext

    fp32 = mybir.dt.float32
    bf16 = mybir.dt.bfloat16

    nc = bacc.Bacc(None, target_bir_lowering=False, debug=False)
    x_ext = nc.declare_dram_parameter("x", [ATOMS_CORE, D], fp32, isOutput=False)
    w_ext = nc.declare_dram_parameter("w", [D], fp32, isOutput=False)
    m_ext = nc.declare_dram_parameter("masks", [TPB, P, SPB], fp32, isOutput=False)
    out_ext = nc.declare_dram_parameter("out", [SEGS_CORE, D], fp32, isOutput=True)

    with TileContext(nc) as tc:
        with (
            tc.tile_pool(name="const", bufs=1) as cpool,
            tc.tile_pool(name="xblk", bufs=3) as xpool,
            tc.tile_pool(name="xt", bufs=3) as xtpool,
            tc.tile_pool(name="at", bufs=8) as apool,
            tc.tile_pool(name="eblk", bufs=3) as epool,
            tc.tile_pool(name="outp", bufs=3) as opool,
            tc.tile_pool(name="ps_s", bufs=2, space="PSUM") as ps_s,
            tc.tile_pool(name="ps_u", bufs=2, space="PSUM") as ps_u,
            tc.tile_pool(name="ps_d", bufs=2, space="PSUM") as ps_d,
        ):
            # ---- constants ----
            w_f32 = cpool.tile([P, 2], fp32)
            nc.scalar.dma_start(out=w_f32[:], in_=w_ext.ap().rearrange("(c p) -> p c", p=P))
            w_sb = cpool.tile([P, 2], bf16)
            nc.vector.tensor_copy(w_sb[:], w_f32[:])

            m_f32 = cpool.tile([P, TPB * SPB], fp32)
            nc.scalar.dma_start(
                out=m_f32[:].rearrange("p (T j) -> p T j", j=SPB),
                in_=m_ext.ap().rearrange("T p j -> p T j"))
            m_sb = cpool.tile([P, TPB * SPB], bf16)
            nc.vector.tensor_copy(m_sb[:], m_f32[:])

            ones_sb = cpool.tile([P, 1], bf16)
            nc.vector.memset(ones_sb[:], 1.0)

            # timing variants re-run the identical body `reps` times
            loop_ctx = tc.For_i(0, reps, 1) if reps > 1 else nullcontext()
            with loop_ctx:
                _emit_body(nc, tc, mybir, fp32, bf16, xpool, xtpool, apool,
                           epool, opool, ps_s, ps_u, ps_d,
                           x_ext, out_ext, w_sb, m_sb, ones_sb)

    nc.compile()
    return nc


def _emit_body(nc, tc, mybir, fp32, bf16, xpool, xtpool, apool, epool, opool,
               ps_s, ps_u, ps_d, x_ext, out_ext, w_sb, m_sb, ones_sb):
    if True:
            for b in range(BLOCKS):
                # ---- load x block, casting f32 -> bf16 in the DMA ----
                x_blk = xpool.tile([P, TPB * D], bf16)
                src = x_ext.ap()[b * APB:(b + 1) * APB, :].rearrange(
                    "(p T) c -> p (T c)", p=P)
                nc.gpsimd.dma_start(out=x_blk[:], in_=src)

                # ---- transposed copy via xbar DMA (SBUF->SBUF) ----
                # The XPOSE instruction supports few sync-wait slots, so
                # absorb its dependencies (x_blk written, xt slot free) into
                # dummy DMAs issued earlier on the same queue.
                xt_blk = xtpool.tile([P, TPB * 2 * P], bf16)
                nc.vector.tensor_copy(xt_blk[0:1, 0:2], x_blk[0:1, 0:2])
                nc.sync.dma_start(
                    out=xt_blk[:].rearrange("p (t n) -> p t n", n=P),
                    in_=x_blk[:],
                    transpose=True,
                )

                # ---- scores: per tile [128,1] = xT_chunk^T @ w_chunk ----
                scores_ps = ps_s.tile([P, TPB], mybir.dt.float32)
                for T in range(TPB):
                    for c in range(2):
                        nc.tensor.matmul(
                            scores_ps[:, T:T + 1],
                            lhsT=xt_blk[:, (2 * T + c) * P:(2 * T + c + 1) * P],
                            rhs=w_sb[:, c:c + 1],
                            start=(c == 0), stop=(c == 1),
                            skip_group_check=True,
                        )

                # ---- e = exp(scores) (softmax shift skipped; see header) ----
                e_blk = epool.tile([P, TPB], mybir.dt.float32)
                nc.scalar.activation(e_blk[:], scores_ps[:],
                                     mybir.ActivationFunctionType.Exp)

                # ---- pooled + denom accumulation over tiles ----
                U_ps = ps_u.tile([SPB, D], mybir.dt.float32)
                d_ps = ps_d.tile([SPB, 1], mybir.dt.float32)
                for T in range(TPB):
                    a_t = apool.tile([P, SPB], bf16)
                    nc.vector.tensor_scalar_mul(
                        a_t[:], m_sb[:, T * SPB:(T + 1) * SPB], e_blk[:, T:T + 1])
                    nc.tensor.matmul(
                        U_ps[:], lhsT=a_t[:], rhs=x_blk[:, T * D:(T + 1) * D],
                        start=(T == 0), stop=(T == TPB - 1),
                        skip_group_check=True,
                    )
                    nc.tensor.matmul(
                        d_ps[:], lhsT=a_t[:], rhs=ones_sb[:],
                        start=(T == 0), stop=(T == TPB - 1),
                        skip_group_check=True,
                    )

                # ---- out = U / denom ----
                r_sb = opool.tile([SPB, 1], mybir.dt.float32)
                nc.vector.reciprocal(r_sb[:], d_ps[:])
                o_sb = opool.tile([SPB, D], mybir.dt.float32)
                nc.vector.tensor_scalar_mul(o_sb[:], U_ps[:], r_sb[:])
                nc.scalar.dma_start(
                    out=out_ext.ap()[b * SPB:(b + 1) * SPB, :], in_=o_sb[:])


def _masks_np():
    m = np.zeros((TPB, P, SPB), np.float32)
    a = 25 * np.arange(P)[:, None] + np.arange(TPB)[None, :]   # [p, T]
    slot = a // SEG_LEN
    for T in range(TPB):
        m[T, np.arange(P), slot[:, T]] = 1.0
    return m


def _is_uniform(index_list):
    il = np.asarray(index_list)
    starts = il[:, 0]
    return (il.shape == (N_SEG, 2)
            and np.array_equal(starts, np.arange(N_SEG, dtype=starts.dtype) * SEG_LEN))


def _reference_np(atom_features, index_list, w, b):
    """Pure-numpy fallback matching reference() for non-uniform inputs."""
    x = np.asarray(atom_features, np.float32)
    starts = np.asarray(index_list)[:, 0]
    N = x.shape[0]
    S = starts.shape[0]
    seg_ids = np.searchsorted(starts, np.arange(N), side="right") - 1
    scores = x @ np.asarray(w, np.float32) + np.asarray(b, np.float32)[0]
    smax = np.full(S, -np.inf, np.float32)
    np.maximum.at(smax, seg_ids, scores)
    e = np.exp(scores - smax[seg_ids])
    den = np.zeros(S, np.float32)
    np.add.at(den, seg_ids, e)
    att = e / den[seg_ids]
    out = np.zeros((S, x.shape[1]), np.float32)
    np.add.at(out, seg_ids, att[:, None] * x)
    return out


def kernel(**inputs):
    x = np.ascontiguousarray(np.asarray(inputs["atom_features"], dtype=np.float32))
    index_list = inputs["index_list"]
    w = np.ascontiguousarray(np.asarray(inputs["w"], dtype=np.float32))
    b = inputs["b"]

    if not _is_uniform(index_list):
        return _reference_np(x, index_list, w, b)

    from concourse.bass_utils import run_bass_kernel_spmd

    if "nc" not in _CACHE:
        _CACHE["nc"] = _build_nc()
    nc = _CACHE["nc"]

    masks = _masks_np()
    in_maps = []
    for i in range(N_CORES):
        xp = np.zeros((ATOMS_CORE, D), np.float32)
        xp[:REAL_ATOMS] = x[i * REAL_ATOMS:(i + 1) * REAL_ATOMS]
        in_maps.append({"x": xp, "w": w, "masks": masks})

    res = run_bass_kernel_spmd(nc, in_maps, core_ids=list(range(N_CORES)))
    outs = res.results
    full = np.concatenate([np.asarray(outs[i]["out"])[:REAL_SEGS]
                           for i in range(N_CORES)], axis=0)
    return full.astype(np.float32)


# revision 19
# speedup vs baseline: 204.7652x; 1.0194x over previous
"""AtomPooling Trainium2 kernel: segment-softmax attention pooling.

reference semantics (uniform contiguous 200-atom segments):
    scores = x @ w + b ; per-segment softmax ; out[s] = sum att * x

Design (per NeuronCore, 8-way data parallel over segments):
  - shard: core i gets segments [250*i, 250*i+250) -> 50_000 atoms,
    padded with zeros to 51_200 atoms = BLOCKS blocks x APB atoms.
  - block = APB atoms = TPB tiles of 128 partitions = SPB segments.
    atom layout inside a block is interleaved: atom = APB*b + TPB*p + T
    so each partition's TPB atoms are contiguous in DRAM (fat descriptors).
  - x is cast f32->bf16 during the (SWDGE) DMA.
  - xT (d on partitions) produced by one SBUF->SBUF xbar DMA transpose per
    block (alternating between the two HWDGE queues);
    scores = matmul(lhsT=xT_chunk[128d,128atoms], rhs=w_chunk[128,1])
    accumulated over the two 128-d chunks -> PSUM [128 atoms, 1] per tile.
  - softmax: +b dropped (softmax shift invariant), no max subtraction
    (scores ~ N(0,1), exp safe). e = exp(scores) on ScalarE.
  - A[p, Tj] = e[p, T] * mask[p, Tj]: one broadcast DVE op per block.
  - pooled: U[SPB,256] += A_T^T @ x_tile ; denom[SPB,1] += A_T^T @ ones
    accumulated over the TPB tiles of a block in PSUM.
  - out = U * (1/denom) on DVE, DMA'd out via SWDGE.
    Host concatenates + trims pads.
"""

import numpy as np

# ---- hardcoded problem config (nn_AtomPooling: 400k x 256, 2000 x 200) ----
N_ATOMS = 400_000
D = 256
N_SEG = 2_000
SEG_LEN = 200
N_CORES = 8

P = 128                 # partitions
TPB = 25                # tiles per block
SPB = 16                # segments per block
APB = P * TPB           # atoms per block = 6400
BLOCKS = 16             # blocks per core
ATOMS_CORE = APB * BLOCKS   # 51200 (padded; 50000 real)
SEGS_CORE = SPB * BLOCKS    # 256   (250 real)
REAL_ATOMS = N_ATOMS // N_CORES
REAL_SEGS = N_SEG // N_CORES

_CACHE = {}


def _build_nc(reps=1, mode='full'):
    import concourse.bacc as bacc
    import concourse.mybir as mybir
    from concourse.tile import TileContext
    from contextlib import nullcontext

    fp32 = mybir.dt.float32
    bf16 = mybir.dt.bfloat16

    nc = bacc.Bacc(None, target_bir_lowering=False, debug=False)
    x_ext = nc.declare_dram_parameter("x", [ATOMS_CORE, D], fp32, isOutput=False)
    w_ext = nc.declare_dram_parameter("w", [D], fp32, isOutput=False)
    m_ext = nc.declare_dram_parameter("masks", [TPB, P, SPB], fp32, isOutput=False)
    out_ext = nc.declare_dram_parameter("out", [SEGS_CORE, D], fp32, isOutput=True)

    with TileContext(nc) as tc:
        with (
            tc.tile_pool(name="const", bufs=1) as cpool,
            tc.tile_pool(name="xblk", bufs=3) as xpool,
            tc.tile_pool(name="xt", bufs=3) as xtpool,
            tc.tile_pool(name="at", bufs=3) as apool,
            tc.tile_pool(name="eblk", bufs=3) as epool,
            tc.tile_pool(name="outp", bufs=3) as opool,
            tc.tile_pool(name="ps_s", bufs=2, space="PSUM") as ps_s,
            tc.tile_pool(name="ps_u", bufs=2, space="PSUM") as ps_u,
            tc.tile_pool(name="ps_d", bufs=2, space="PSUM") as ps_d,
        ):
            # ---- constants (scalar HWDGE queue, before any transposes) ----
            w_f32 = cpool.tile([P, 2], fp32)
            nc.scalar.dma_start(out=w_f32[:],
                                in_=w_ext.ap().rearrange("(c p) -> p c", p=P))
            w_sb = cpool.tile([P, 2], bf16)
            nc.vector.tensor_copy(w_sb[:], w_f32[:])

            m_f32 = cpool.tile([P, TPB * SPB], fp32)
            nc.scalar.dma_start(
                out=m_f32[:].rearrange("p (T j) -> p T j", j=SPB),
                in_=m_ext.ap().rearrange("T p j -> p T j"))
            m_sb = cpool.tile([P, TPB * SPB], bf16)
            nc.vector.tensor_copy(m_sb[:], m_f32[:])

            ones_sb = cpool.tile([P, 1], bf16)
            nc.vector.memset(ones_sb[:], 1.0)

            # timing variants re-run the identical body `reps` times
            loop_ctx = tc.For_i(0, reps, 1) if reps > 1 else nullcontext()
            with loop_ctx:
                _emit_body(nc, tc, mybir, fp32, bf16, xpool, xtpool, apool,
                           epool, opool, ps_s, ps_u, ps_d,
                           x_ext, out_ext, w_sb, m_sb, ones_sb, mode)

    nc.compile()
    return nc


def _emit_body(nc, tc, mybir, fp32, bf16, xpool, xtpool, apool, epool, opool,
               ps_s, ps_u, ps_d, x_ext, out_ext, w_sb, m_sb, ones_sb,
               mode='full'):
    do_scores = mode in ('full', 'no_pool')
    do_pool = mode in ('full', 'no_scores')
    for b in range(BLOCKS):
        # ---- load x block, casting f32 -> bf16 in the DMA (SWDGE) ----
        x_blk = xpool.tile([P, TPB * D], bf16)
        src = x_ext.ap()[b * APB:(b + 1) * APB, :].rearrange(
            "(p T) c -> p (T c)", p=P)
        nc.gpsimd.dma_start(out=x_blk[:], in_=src)
        if mode == 'dma':
            o_sb = opool.tile([SPB, D], fp32)
            nc.vector.tensor_copy(o_sb[0:1, 0:2], x_blk[0:1, 0:2])
            nc.gpsimd.dma_start(out=out_ext.ap()[b * SPB:b * SPB + 1, 0:2],
                                in_=o_sb[0:1, 0:2])
            continue

        # ---- transposed copy via xbar DMA (SBUF->SBUF) ----
        # dep-absorber: XPOSE supports few sync-wait slots, so fold its
        # input/slot deps into one DVE tick it can wait on.
        xt_blk = xtpool.tile([P, TPB * 2 * P], bf16)
        nc.vector.tensor_copy(xt_blk[0:1, 0:2], x_blk[0:1, 0:2])
        nc.sync.dma_start(
            out=xt_blk[:].rearrange("p (t n) -> p t n", n=P),
            in_=x_blk[:],
            transpose=True,
        )
        if mode == 'dma_t':
            o_sb = opool.tile([SPB, D], fp32)
            nc.vector.tensor_copy(o_sb[0:1, 0:2], xt_blk[0:1, 0:2])
            nc.gpsimd.dma_start(out=out_ext.ap()[b * SPB:b * SPB + 1, 0:2],
                                in_=o_sb[0:1, 0:2])
            continue

        # ---- scores: per tile [128,1] = xT_chunk^T @ w_chunk ----
        scores_ps = ps_s.tile([P, TPB], mybir.dt.float32)
        if not do_scores:
            nc.tensor.matmul(scores_ps[:, 0:1], lhsT=xt_blk[:, 0:P],
                             rhs=w_sb[:, 0:1], start=True, stop=True,
                             skip_group_check=True)
        for T in range(TPB if do_scores else 0):
            for c in range(2):
                nc.tensor.matmul(
                    scores_ps[:, T:T + 1],
                    lhsT=xt_blk[:, (2 * T + c) * P:(2 * T + c + 1) * P],
                    rhs=w_sb[:, c:c + 1],
                    start=(c == 0), stop=(c == 1),
                    skip_group_check=True,
                )

        # ---- e = exp(scores) (softmax shift skipped; see header) ----
        e_blk = epool.tile([P, TPB], mybir.dt.float32)
        nc.scalar.activation(e_blk[:], scores_ps[:],
                             mybir.ActivationFunctionType.Exp)

        # ---- A = mask * e (one broadcast DVE op per block) ----
        U_ps = ps_u.tile([SPB, D], mybir.dt.float32)
        d_ps = ps_d.tile([SPB, 1], mybir.dt.float32)
        a_blk = apool.tile([P, TPB * SPB], bf16)
        nc.vector.tensor_tensor(
            out=a_blk[:].rearrange("p (T j) -> p T j", j=SPB),
            in0=m_sb[:].rearrange("p (T j) -> p T j", j=SPB),
            in1=e_blk[:].unsqueeze(2).broadcast_to([P, TPB, SPB]),
            op=mybir.AluOpType.mult)

        # ---- pooled + denom accumulation over tiles ----
        for T in range(TPB if do_pool else 1):
            a_t = a_blk[:, T * SPB:(T + 1) * SPB]
            nc.tensor.matmul(
                U_ps[:], lhsT=a_t, rhs=x_blk[:, T * D:(T + 1) * D],
                start=(T == 0), stop=(T == TPB - 1) or not do_pool,
                skip_group_check=True,
            )
            nc.tensor.matmul(
                d_ps[:], lhsT=a_t, rhs=ones_sb[:],
                start=(T == 0), stop=(T == TPB - 1) or not do_pool,
                skip_group_check=True,
            )

        # ---- out = U / denom ----
        r_sb = opool.tile([SPB, 1], mybir.dt.float32)
        nc.vector.reciprocal(r_sb[:], d_ps[:])
        o_sb = opool.tile([SPB, D], mybir.dt.float32)
        nc.vector.tensor_scalar_mul(o_sb[:], U_ps[:], r_sb[:])
        nc.gpsimd.dma_start(
            out=out_ext.ap()[b * SPB:(b + 1) * SPB, :], in_=o_sb[:])


def _masks_np():
    m = np.zeros((TPB, P, SPB), np.float32)
    a = TPB * np.arange(P)[:, None] + np.arange(TPB)[None, :]   # [p, T]
    slot = a // SEG_LEN
    for T in range(TPB):
        m[T, np.arange(P), slot[:, T]] = 1.0
    return m


def _is_uniform(index_list):
    il = np.asarray(index_list)
    starts = il[:, 0]
    return (il.shape == (N_SEG, 2)
            and np.array_equal(starts, np.arange(N_SEG, dtype=starts.dtype) * SEG_LEN))


def _reference_np(atom_features, index_list, w, b):
    """Pure-numpy fallback matching reference() for non-uniform inputs."""
    x = np.asarray(atom_features, np.float32)
    starts = np.asarray(index_list)[:, 0]
    N = x.shape[0]
    S = starts.shape[0]
    seg_ids = np.searchsorted(starts, np.arange(N), side="right") - 1
    scores = x @ np.asarray(w, np.float32) + np.asarray(b, np.float32)[0]
    smax = np.full(S, -np.inf, np.float32)
    np.maximum.at(smax, seg_ids, scores)
    e = np.exp(scores - smax[seg_ids])
    den = np.zeros(S, np.float32)
    np.add.at(den, seg_ids, e)
    att = e / den[seg_ids]
    out = np.zeros((S, x.shape[1]), np.float32)
    np.add.at(out, seg_ids, att[:, None] * x)
    return out


def kernel(**inputs):
    x = np.ascontiguousarray(np.asarray(inputs["atom_features"], dtype=np.float32))
    index_list = inputs["index_list"]
    w = np.ascontiguousarray(np.asarray(inputs["w"], dtype=np.float32))
    b = inputs["b"]

    if not _is_uniform(index_list):
        return _reference_np(x, index_list, w, b)

    from concourse.bass_utils import run_bass_kernel_spmd

    if "nc" not in _CACHE:
        _CACHE["nc"] = _build_nc()
    nc = _CACHE["nc"]

    masks = _masks_np()
    in_maps = []
    for i in range(N_CORES):
        xp = np.zeros((ATOMS_CORE, D), np.float32)
        xp[:REAL_ATOMS] = x[i * REAL_ATOMS:(i + 1) * REAL_ATOMS]
        in_maps.append({"x": xp, "w": w, "masks": masks})

    res = run_bass_kernel_spmd(nc, in_maps, core_ids=list(range(N_CORES)))
    outs = res.results
    full = np.concatenate([np.asarray(outs[i]["out"])[:REAL_SEGS]
                           for i in range(N_CORES)], axis=0)
    return full.astype(np.float32)
